# revision 1
# baseline (speedup 1.0000x reference)
import numpy as np
import concourse.bacc as bacc
import concourse.mybir as mybir
from concourse.tile import TileContext
from concourse.bass_utils import run_bass_kernel_spmd

DIM_INPUT = 128
DIM_REC = 512
DIM_OUT = 256
BATCH = 512
NCORES = 8
B = BATCH // NCORES  # 64 per-core batch
T = DIM_INPUT        # 128 timesteps
KJ = DIM_REC // 128  # 4 chunks of the recurrent dim
OJ = DIM_OUT // 128  # 2 chunks of the output dim

F32 = mybir.dt.float32
MMDT = mybir.dt.float16  # matmul operand dtype (FWL + 1 cyc/row on PE)
MMNP = np.float16

# MM issue order within a step. 's{j}' is the x-projection matmul for
# group j (start=True seeds psum bank j); (j,k) accumulates Wh[k->j]@g_k.
# Order from discrete-event search over the epilogue dependency chain
# (sched_search.py, scalar handles groups 0,1 / vector 2,3); model
# period 915ns vs 1003ns for the previous hand schedule.
STEP_ORDER = ['s1', 's2', 's0', 's3', (3, 0), (2, 0), (0, 2), (0, 0),
              (1, 2), (2, 2), (0, 3), (0, 1), (2, 3), (2, 1), (1, 3),
              (1, 0), (1, 1), (3, 3), (3, 1), (3, 2)]


def _build_nc():
    nc = bacc.Bacc("TRN2", target_bir_lowering=False, debug=False,
                   num_devices=NCORES)
    xT = nc.dram_tensor("xT", [DIM_INPUT, B], MMDT, kind="ExternalInput")
    WhT = nc.dram_tensor("WhT", [DIM_REC, DIM_REC], MMDT, kind="ExternalInput")
    WxT = nc.dram_tensor("WxT", [DIM_INPUT, DIM_REC], MMDT, kind="ExternalInput")
    whyR = nc.dram_tensor("whyR", [128, KJ * DIM_OUT], MMDT, kind="ExternalInput")
    bcR = nc.dram_tensor("bcR", [128, KJ], F32, kind="ExternalInput")
    byR = nc.dram_tensor("byR", [128, OJ], F32, kind="ExternalInput")
    yT = nc.dram_tensor("yT", [DIM_OUT, B], F32, kind="ExternalOutput")

    RELU = mybir.ActivationFunctionType.Relu
    IDENT = mybir.ActivationFunctionType.Identity
    ADD = mybir.AluOpType.add
    MAX = mybir.AluOpType.max

    with TileContext(nc) as tc:
        with tc.tile_pool(name="w", bufs=1) as wp, \
             tc.tile_pool(name="s", bufs=1) as sp, \
             tc.psum_pool(name="p", bufs=1) as pp:
            wh = [wp.tile([128, DIM_REC], MMDT, name=f"wh{k}") for k in range(KJ)]
            wx = wp.tile([128, DIM_REC], MMDT, name="wx")
            whyt = wp.tile([128, KJ * DIM_OUT], MMDT, name="why")
            bct = wp.tile([128, KJ], F32, name="bct")
            byt = wp.tile([128, OJ], F32, name="byt")
            xt = sp.tile([128, B], MMDT, name="xt")
            g = [[sp.tile([128, B], MMDT, name=f"g{p}_{k}") for k in range(KJ)]
                 for p in range(2)]
            ps = [[pp.tile([128, B], F32, name=f"ps{p}_{j}") for j in range(KJ)]
                  for p in range(2)]
            psy = [ps[0][0], ps[0][1]]  # reuse phase-0 banks (free after step T-1)

            # startup DMAs: big weight loads split across the three
            # DMA-capable queues; tail-only tensors (why/byt) trail on sync.
            nc.sync.dma_start(out=xt[:], in_=xT[:])
            nc.sync.dma_start(out=wx[0:64, :], in_=WxT[0:64, :])
            nc.scalar.dma_start(out=wx[64:128, :], in_=WxT[64:128, :])
            nc.sync.dma_start(out=bct[:], in_=bcR[:])
            nc.gpsimd.dma_start(out=wh[0][:], in_=WhT[0:128, :])
            nc.scalar.dma_start(out=wh[1][:], in_=WhT[128:256, :])
            nc.gpsimd.dma_start(out=wh[2][:], in_=WhT[256:384, :])
            nc.scalar.dma_start(out=wh[3][:], in_=WhT[384:512, :])
            nc.sync.dma_start(out=whyt[:], in_=whyR[:])
            nc.sync.dma_start(out=byt[:], in_=byR[:])

            def epilogue(dst, psrc):
                # dst_j = relu(psum_j + bc_j); scalar takes 0,1 / vector 2,3
                nc.scalar.activation(dst[0][:], psrc[0][:], RELU,
                                     bias=bct[:, 0:1])
                nc.scalar.activation(dst[1][:], psrc[1][:], RELU,
                                     bias=bct[:, 1:2])
                nc.vector.tensor_scalar(dst[2][:], psrc[2][:],
                                        bct[:, 2:3], 0.0, ADD, MAX)
                nc.vector.tensor_scalar(dst[3][:], psrc[3][:],
                                        bct[:, 3:4], 0.0, ADD, MAX)

            # step 1 (h0 = 0): g0_j = relu((x @ W_x2h.T).T[j] + bc[j])
            for j in range(KJ):
                nc.tensor.matmul(ps[0][j][:], wx[:, j * 128:(j + 1) * 128],
                                 xt[:], start=True, stop=True)
            epilogue(g[0], ps[0])

            # 127 recurrent steps: g' = relu(x @ Wx + Wh @ g + bc)
            for s in range(1, T):
                cur, nxt = g[(s + 1) % 2], g[s % 2]
                pcur = ps[s % 2]
                grp = [0] * KJ
                for it in STEP_ORDER:
                    if isinstance(it, str):
                        j = int(it[1])
                        nc.tensor.matmul(pcur[j][:],
                                         wx[:, j * 128:(j + 1) * 128],
                                         xt[:], start=True, stop=False)
                    else:
                        j, k = it
                        grp[j] += 1
                        nc.tensor.matmul(pcur[j][:],
                                         wh[k][:, j * 128:(j + 1) * 128],
                                         cur[k][:], start=False,
                                         stop=(grp[j] == KJ))
                epilogue(nxt, pcur)

            gfin = g[(T - 1) % 2]
            # yT[jslice] = W_h2y[jslice] @ h.T + b_h2y[jslice]
            for j in range(OJ):
                for k in range(KJ):
                    nc.tensor.matmul(
                        psy[j][:],
                        whyt[:, k * DIM_OUT + j * 128:k * DIM_OUT + (j + 1) * 128],
                        gfin[k][:], start=(k == 0), stop=(k == KJ - 1))
            ytile = [sp.tile([128, B], F32, name=f"yt{j}") for j in range(OJ)]
            nc.scalar.activation(ytile[0][:], psy[0][:], IDENT,
                                 bias=byt[:, 0:1])
            nc.vector.tensor_scalar(ytile[1][:], psy[1][:], byt[:, 1:2],
                                    None, ADD)
            nc.sync.dma_start(out=yT[0:128, :], in_=ytile[0][:])
            nc.gpsimd.dma_start(out=yT[128:256, :], in_=ytile[1][:])

    nc.compile()
    return nc


_NC = None
TRACE = False
TRACE_TMPDIR = None
LAST_RESULTS = None


def kernel(x, W_x2h, b_x2h, W_h2h, b_h2h, W_h2y, b_h2y):
    global _NC, LAST_RESULTS
    if _NC is None:
        _NC = _build_nc()

    x = np.asarray(x, np.float32)
    WhyT = np.asarray(W_h2y, np.float32).T.astype(MMNP)
    bc = np.asarray(b_x2h, np.float32) + np.asarray(b_h2h, np.float32)
    shared = {
        "WhT": np.ascontiguousarray(np.asarray(W_h2h, np.float32).T.astype(MMNP)),
        "WxT": np.ascontiguousarray(np.asarray(W_x2h, np.float32).T.astype(MMNP)),
        "whyR": np.ascontiguousarray(np.concatenate(
            [WhyT[k * 128:(k + 1) * 128, :] for k in range(KJ)], axis=1)),
        "bcR": np.ascontiguousarray(bc.reshape(KJ, 128).T),
        "byR": np.ascontiguousarray(
            np.asarray(b_h2y, np.float32).reshape(OJ, 128).T),
    }
    ins = []
    for i in range(NCORES):
        m = dict(shared)
        m["xT"] = np.ascontiguousarray(x[i * B:(i + 1) * B, :].T.astype(MMNP))
        ins.append(m)

    kw = {}
    if TRACE:
        kw = {"trace": True, "tmpdir": TRACE_TMPDIR}
    res = run_bass_kernel_spmd(_NC, ins, core_ids=list(range(NCORES)), **kw)
    LAST_RESULTS = res
    out = np.empty((BATCH, DIM_OUT), np.float32)
    for i in range(NCORES):
        out[i * B:(i + 1) * B, :] = res.results[i]["yT"].T
    return out

